# revision 40
# baseline (speedup 1.0000x reference)
"""Trainium2 Bass kernel for masked-softmax attention (sparse_attention).

reference:
    S = Q @ K^T / sqrt(128)            # [N, nq, nk]
    A = softmax(S, axis=2)
    A = A * mask;  A = A / (sum_k A + 1e-6)
    O = A @ V

Device identity (softmax normalizer and any constant mask scale cancel in
the renormalization):
    E = exp(S); P = E * (mask*255)
    O[q, :] = (P @ V)[q, :] / sum_k P[q, k]

Sharding: N=32 batch-heads split across 8 NeuronCores, 4 per core; no
cross-core communication. Host staging: Q/K transposed to [d, n] bf16 with
Q pre-scaled by 1/(sqrt(d)*16) (so PSUM scores are t/16 — the activation
rescales by 16 and the poly-exp path consumes t/16 directly), V tiled with
a ones column appended (P @ [V|1] yields the renorm denominator inside
mm2), mask transposed to [k, q] and split per k-tile between bf16*255 and
u8 copies.

The exponential is the scarce resource (ACT does 1 elem/cycle @1.2GHz =
109us/core just for exp, vs PE's 110us of matmul): per (batch, q-half)
slab the 16 k-tiles are split across engines so every engine lands at
~110us/core:
  7 tiles: ACT exp -> DVE mult (bf16 mask, DVE 2x mode)
  6 tiles: ACT exp -> Pool (gpsimd) mult (u8 mask)
  3 tiles: custom-DVE exp: cubic poly of t/16, then (.)^16 * mask fused
           (two 1-pass custom DVE ops, skips ACT entirely)

Per-core pipeline, per (batch b, q-half h of 1024):
  k-phase, per k-tile kt: mm1 (PE) -> exp -> mask-mult -> P^T slab
  q-phase (interleaved into the next k-phase at kt 7..14), per q-tile:
    mm2 (PE): O|denom = sum_kt PT[kt].T @ [V_kt|1] -> PSUM
    recip+scale (DVE): st = O * (1/denom)
  store st -> out in 4-q-tile chunks on the ACT HWDGE ring.
"""
import sys

sys.path.insert(0, "/opt/trn_rl_repo")

import ml_dtypes
import numpy as np

from concourse import bacc, mybir, tile
from concourse.bass_utils import run_bass_kernel_spmd

N, NQ, NK, D = 32, 2048, 2048, 128
N_CORES = 8
B = N // N_CORES          # batches per core
QT_TILES = NQ // 128      # q tiles per batch
KT_TILES = NK // 128      # k tiles per batch
QH = NQ // 2              # q-half width
SCALE = float(1.0 / np.sqrt(D))

# cubic minimax fit of e^y on [-0.5375, 0.5375] with p(y)=1+y+y^2*(B2+A3*y)
A3 = 0.17059872676988808
B2 = 0.5101347134234719

# per-slab k-tile engine assignment
KT_CUSTOM = (3, 9, 13)                      # custom-DVE exp^16 path (u8 mask)
KT_POOL = (1, 5, 7, 8, 11, 15)              # ACT exp -> Pool mult (u8 mask)
KT_DVE = (0, 2, 4, 6, 10, 12, 14)           # ACT exp -> DVE mult (bf16 mask)
KT_U8 = tuple(sorted(KT_CUSTOM + KT_POOL))  # u8-mask tiles, in kt order
NB_T = len(KT_DVE)
NU_T = len(KT_U8)
U8_IDX = {kt: i for i, kt in enumerate(KT_U8)}
BF_IDX = {kt: i for i, kt in enumerate(KT_DVE)}

F32 = mybir.dt.float32
BF16 = mybir.dt.bfloat16
U8 = mybir.dt.uint8

_cached = {}


def _register_dve_ops():
    """Register the two custom DVE ops (runtime extension of dve_ops.OPS).

    EXP16C: w = 1 + y + y^2*(C2 + C1*y)  ~= exp(y) for y = t/16 (6 ALU stages)
    SQ16M:  out = (w^16) * mask          (4 squarings + mult, 5 stages)
    """
    import concourse.dve_ops as dops
    from concourse.dve_ops import DveOp
    from concourse.dve_spec import C1, C2, One, Spec, Src0, Src1, _has_src1, lower, sq
    from concourse.dve_uop import DveOpSpec

    def register(name, spec):
        if name in dops._SUB_OPCODE_FOR_NAME:
            return next(o for o in dops.OPS if o.name == name)
        row = max(dops._SUB_OPCODE_FOR_NAME.values()) + 1
        dops._SUB_OPCODE_FOR_NAME[name] = row
        shas = {}
        for ver in ("v3", "v4"):
            uops = lower(spec, ver=ver)
            shas[ver] = DveOpSpec(
                name=name, opcode=row, uops=uops, rd1_en=_has_src1(spec)
            ).sha(ver)
        op = DveOp(name, spec, subdim=False, uops_sha=shas)
        dops.OPS.append(op)
        dops.CUSTOM_DVE_SPECS[name] = spec
        return op

    exp16c = register(
        "EXP16C",
        Spec(
            body=(sq(Src0) * (Src0 * C1 + C2)) + Src0 + One,
            reference=lambda in0, in1, s0, s1, imm2: (
                1.0 + in0 + in0 * in0 * (imm2 + s1 * in0)
            ),
        ),
    )
    sq16m = register(
        "SQ16M",
        Spec(
            body=sq(sq(sq(sq(Src0)))) * Src1,
            reference=lambda in0, in1, s0, s1, imm2: (
                in0.astype(np.float32) ** 16
            )
            * in1,
        ),
    )
    return exp16c, sq16m


def build():
    if "nc" in _cached:
        return _cached["nc"]
    exp16c, sq16m = _register_dve_ops()
    nc = bacc.Bacc("TRN2", target_bir_lowering=False, debug=False)

    qt_d = nc.dram_tensor("queriesT", [B, D, NQ], BF16, kind="ExternalInput").ap()
    kt_d = nc.dram_tensor("keysT", [B, D, NK], BF16, kind="ExternalInput").ap()
    v_d = nc.dram_tensor(
        "valuesP", [B, 128, KT_TILES, D + 1], BF16, kind="ExternalInput"
    ).ap()
    mb_d = nc.dram_tensor(
        "maskB", [B, 2, 128, NB_T, QH], BF16, kind="ExternalInput"
    ).ap()
    mu_d = nc.dram_tensor(
        "maskU", [B, 2, 128, NU_T, QH], U8, kind="ExternalInput"
    ).ap()
    o_d = nc.dram_tensor("out", [B, 128, QT_TILES, D], BF16, kind="ExternalOutput").ap()

    with tile.TileContext(nc) as tc:
        with (
            tc.tile_pool(name="tr", bufs=3) as trpool,
            tc.tile_pool(name="vbo", bufs=3) as vpool,
            tc.tile_pool(name="maskc", bufs=3) as mpool,
            tc.tile_pool(name="work", bufs=6) as wpool,
            tc.tile_pool(name="wexp", bufs=2) as wepool,
            tc.tile_pool(name="ptslab", bufs=2) as ptpool,
            tc.tile_pool(name="stage", bufs=3) as stpool,
            tc.tile_pool(name="spsum", bufs=3, space="PSUM") as spool,
            tc.tile_pool(name="opsum", bufs=2, space="PSUM") as opool,
        ):
            def q_iter(prev, qc, tail=False):
                """One q-tile of the q-phase for a finished P^T slab."""
                pt, vb, st, b, h, qlo, nq = prev
                qtile = (h * QH + qlo) // 128 + qc
                if tail and qc % 2 == 0:
                    # the s-pool's PSUM banks are free once the last k-phase
                    # is done; alternating rings gives the tail 5 slots
                    o_full = spool.tile([128, QH], F32, tag="s", name="o_tail")
                    o_ps = o_full[:, 0:D + 1]
                else:
                    o_ps = opool.tile([128, D + 1], F32, tag="o")
                for kt in range(KT_TILES):
                    nc.tensor.matmul(
                        o_ps[:],
                        pt[:, kt, qc * 128:(qc + 1) * 128],
                        vb[:, kt, :],
                        start=(kt == 0),
                        stop=(kt == KT_TILES - 1),
                    )
                rd = wpool.tile([128, 1], F32, tag="rd")
                nc.vector.reciprocal(rd[:], o_ps[:, D:D + 1])
                nc.vector.tensor_scalar_mul(st[:, qtile, :], o_ps[:, 0:D], rd[:])
                if qtile % 4 == 3:
                    nc.scalar.dma_start(
                        o_d[b, :, qtile - 3:qtile + 1, :],
                        st[:, qtile - 3:qtile + 1, :],
                    )

            # slabs: (b, h, qlo, qw) — one per (batch, q-half)
            slabs = [(b, h, 0, QH) for b in range(B) for h in range(2)]

            prev = None
            vb = st = kta = ktb = ktc = None
            qth = [None, None]
            for b, h, qlo, qw in slabs:
                if h == 0 and qlo == 0:
                    # per-batch prologue; DMA emission order == consumption
                    # order (serial DMA pipe)
                    kta = trpool.tile([128, 256], BF16, tag="kta")
                    ktb = trpool.tile([128, 768], BF16, tag="ktb")
                    ktc = trpool.tile([128, 1024], BF16, tag="ktc")
                    qth = [
                        trpool.tile([128, QH], BF16, tag=f"qth{hh}", name=f"qth{hh}")
                        for hh in range(2)
                    ]
                    nc.sync.dma_start(kta[:], kt_d[b, :, 0:256])
                    nc.sync.dma_start(qth[0][:], qt_d[b, :, 0:QH])
                    vb = vpool.tile([128, KT_TILES, D + 1], BF16, tag="vb")
                    st = stpool.tile([128, QT_TILES, D], BF16, tag="st")

                qsl = slice(qlo, qlo + qw)
                mba = mpool.tile([128, 2, QH], BF16, tag="mba")
                mbb = mpool.tile([128, NB_T - 2, QH], BF16, tag="mbb")
                mua = mpool.tile([128, 2, QH], U8, tag="mua")
                mub = mpool.tile([128, NU_T - 2, QH], U8, tag="mub")
                nc.sync.dma_start(mba[:, :, 0:qw], mb_d[b, h, :, 0:2, qsl])
                nc.sync.dma_start(mua[:, :, 0:qw], mu_d[b, h, :, 0:2, qsl])
                if h == 0 and qlo == 0:
                    nc.sync.dma_start(ktb[:], kt_d[b, :, 256:1024])
                nc.sync.dma_start(mbb[:, :, 0:qw], mb_d[b, h, :, 2:NB_T, qsl])
                if h == 0 and qlo == 0:
                    nc.sync.dma_start(ktc[:], kt_d[b, :, 1024:NK])
                    nc.sync.dma_start(qth[1][:], qt_d[b, :, QH:NQ])
                    nc.sync.dma_start(vb[:], v_d[b])
                nc.sync.dma_start(mub[:, :, 0:qw], mu_d[b, h, :, 2:NU_T, qsl])

                def mask_bf(kt):
                    i = BF_IDX[kt]
                    return mba[:, i, 0:qw] if i < 2 else mbb[:, i - 2, 0:qw]

                def mask_u8(kt, three_d=False):
                    i = U8_IDX[kt]
                    t_ = mua if i < 2 else mub
                    i_ = i if i < 2 else i - 2
                    if three_d:
                        return t_[:, i_:i_ + 1, 0:qw]
                    return t_[:, i_, 0:qw]

                # interleave slots for the previous slab's q-phase: pack into
                # the back half so a late pt slab can't stall the PE queue
                if prev is not None:
                    p_nq = prev[6]
                    if p_nq == 8:
                        qslot = {7 + j: j for j in range(8)}
                    else:
                        qslot = {8 + 2 * j: j for j in range(p_nq)}
                else:
                    qslot = {}

                pt = ptpool.tile([128, KT_TILES, QH], BF16, tag="pt")
                for kt in range(KT_TILES):
                    s_ps = spool.tile([128, QH], F32, tag="s")
                    for c in range(qw // 512):
                        nc.tensor.matmul(
                            s_ps[:, c * 512:(c + 1) * 512],
                            kta[:, kt * 128:(kt + 1) * 128]
                            if kt < 2
                            else (
                                ktb[:, (kt - 2) * 128:(kt - 1) * 128]
                                if kt < 8
                                else ktc[:, (kt - 8) * 128:(kt - 7) * 128]
                            ),
                            qth[h][:, qlo + c * 512:qlo + (c + 1) * 512],
                            start=True,
                            stop=True,
                        )
                    if kt in KT_CUSTOM:
                        w_sb = wepool.tile([128, QH], F32, tag="w")
                        nc.vector._custom_dve(
                            exp16c,
                            out=w_sb[:, 0:qw],
                            in0=s_ps[:, 0:qw],
                            s1=A3,
                            imm2=B2,
                        )
                        nc.vector._custom_dve(
                            sq16m,
                            out=pt[:, kt, 0:qw],
                            in0=w_sb[:, 0:qw],
                            in1=mask_u8(kt, three_d=True),
                        )
                    else:
                        e_sb = wpool.tile([128, QH], BF16, tag="e")
                        nc.scalar.activation(
                            e_sb[:, 0:qw],
                            s_ps[:, 0:qw],
                            mybir.ActivationFunctionType.Exp,
                            scale=16.0,
                        )
                        if kt in KT_DVE:
                            nc.vector.tensor_tensor(
                                out=pt[:, kt, 0:qw],
                                in0=e_sb[:, 0:qw],
                                in1=mask_bf(kt),
                                op=mybir.AluOpType.mult,
                            )
                        else:
                            nc.gpsimd.tensor_tensor(
                                out=pt[:, kt, 0:qw],
                                in0=e_sb[:, 0:qw],
                                in1=mask_u8(kt),
                                op=mybir.AluOpType.mult,
                            )
                    if kt in qslot:
                        q_iter(prev, qslot[kt])
                prev = (pt, vb, st, b, h, qlo, qw // 128)

            for qc in range(prev[6]):
                q_iter(prev, qc, tail=True)

    nc.compile()
    _cached["nc"] = nc
    return nc


def kernel(queries, keys, values, mask, _trace=False, **kw):
    queries = np.asarray(queries, dtype=np.float32)
    keys = np.asarray(keys, dtype=np.float32)
    values = np.asarray(values, dtype=np.float32)
    mask = np.asarray(mask, dtype=np.float32)
    nc = build()
    in_maps = []
    for c in range(N_CORES):
        sl = slice(c * B, (c + 1) * B)
        # [V | 1]: ones column so P @ [V|1] emits the denominator
        vp = np.ones((B, KT_TILES, 128, D + 1), dtype=np.float32)
        vp[:, :, :, :D] = values[sl].reshape(B, KT_TILES, 128, D)
        # mask^T packed per (b, h): [B, 2, 128, KT, QH]; then split per
        # k-tile into bf16*255 and u8 copies (one common 255 scale — it
        # cancels in the renormalization)
        mt = (
            mask[sl]
            .transpose(0, 2, 1)  # [B, k, q]
            .reshape(B, KT_TILES, 128, 2, QH)
            .transpose(0, 3, 2, 1, 4)  # [B, 2, 128p, KT, QH]
        )
        mb = (mt[:, :, :, KT_DVE, :] * 255.0).astype(ml_dtypes.bfloat16)
        mu = np.rint(mt[:, :, :, KT_U8, :] * 255.0).astype(np.uint8)
        in_maps.append(
            {
                "queriesT": np.ascontiguousarray(
                    queries[sl].transpose(0, 2, 1) * (SCALE / 16.0)
                ).astype(ml_dtypes.bfloat16),
                "keysT": np.ascontiguousarray(
                    keys[sl].transpose(0, 2, 1)
                ).astype(ml_dtypes.bfloat16),
                "valuesP": np.ascontiguousarray(
                    vp.transpose(0, 2, 1, 3)
                ).astype(ml_dtypes.bfloat16),
                "maskB": np.ascontiguousarray(mb),
                "maskU": np.ascontiguousarray(mu),
            }
        )
    res = run_bass_kernel_spmd(
        nc, in_maps, core_ids=list(range(N_CORES)), trace=_trace
    )
    out = np.concatenate(
        [
            res.results[c]["out"]
            .astype(np.float32)
            .transpose(0, 2, 1, 3)
            .reshape(B, NQ, D)
            for c in range(N_CORES)
        ],
        axis=0,
    )
    if _trace:
        return out, res
    return out


# revision 41
# speedup vs baseline: 1.0026x; 1.0026x over previous
"""Trainium2 Bass kernel for masked-softmax attention (sparse_attention).

reference:
    S = Q @ K^T / sqrt(128)            # [N, nq, nk]
    A = softmax(S, axis=2)
    A = A * mask;  A = A / (sum_k A + 1e-6)
    O = A @ V

Device identity (softmax normalizer and any constant mask scale cancel in
the renormalization):
    E = exp(S); P = E * (mask*255)
    O[q, :] = (P @ V)[q, :] / sum_k P[q, k]

Sharding: N=32 batch-heads split across 8 NeuronCores, 4 per core; no
cross-core communication. Host staging: Q/K transposed to [d, n] bf16 with
Q pre-scaled by 1/(sqrt(d)*16) (so PSUM scores are t/16 — the activation
rescales by 16 and the poly-exp path consumes t/16 directly), V tiled with
a ones column appended (P @ [V|1] yields the renorm denominator inside
mm2), mask transposed to [k, q] and split per k-tile between bf16*255 and
u8 copies.

The exponential is the scarce resource (ACT does 1 elem/cycle @1.2GHz =
109us/core just for exp, vs PE's 110us of matmul): per (batch, q-half)
slab the 16 k-tiles are split across engines so every engine lands at
~110us/core:
  7 tiles: ACT exp -> DVE mult (bf16 mask, DVE 2x mode)
  6 tiles: ACT exp -> Pool (gpsimd) mult (u8 mask)
  3 tiles: custom-DVE exp: cubic poly of t/16, then (.)^16 * mask fused
           (two 1-pass custom DVE ops, skips ACT entirely)

Per-core pipeline, per (batch b, q-half h of 1024):
  k-phase, per k-tile kt: mm1 (PE) -> exp -> mask-mult -> P^T slab
  q-phase (interleaved into the next k-phase at kt 7..14), per q-tile:
    mm2 (PE): O|denom = sum_kt PT[kt].T @ [V_kt|1] -> PSUM
    recip+scale (DVE): st = O * (1/denom)
  store st -> out in 4-q-tile chunks on the ACT HWDGE ring.
"""
import sys

sys.path.insert(0, "/opt/trn_rl_repo")

import ml_dtypes
import numpy as np

from concourse import bacc, mybir, tile
from concourse.bass_utils import run_bass_kernel_spmd

N, NQ, NK, D = 32, 2048, 2048, 128
N_CORES = 8
B = N // N_CORES          # batches per core
QT_TILES = NQ // 128      # q tiles per batch
KT_TILES = NK // 128      # k tiles per batch
QH = NQ // 2              # q-half width
SCALE = float(1.0 / np.sqrt(D))

# cubic minimax fit of e^y on [-0.5375, 0.5375] with p(y)=1+y+y^2*(B2+A3*y)
A3 = 0.17059872676988808
B2 = 0.5101347134234719

# per-slab k-tile engine assignment
KT_CUSTOM = (3, 9, 13)                      # custom-DVE exp^16 path (u8 mask)
KT_POOL = (1, 5, 7, 8, 11, 15)              # ACT exp -> Pool mult (u8 mask)
KT_DVE = (0, 2, 4, 6, 10, 12, 14)           # ACT exp -> DVE mult (bf16 mask)
KT_U8 = tuple(sorted(KT_CUSTOM + KT_POOL))  # u8-mask tiles, in kt order
NB_T = len(KT_DVE)
NU_T = len(KT_U8)
U8_IDX = {kt: i for i, kt in enumerate(KT_U8)}
BF_IDX = {kt: i for i, kt in enumerate(KT_DVE)}

F32 = mybir.dt.float32
BF16 = mybir.dt.bfloat16
U8 = mybir.dt.uint8

_cached = {}


def _register_dve_ops():
    """Register the two custom DVE ops (runtime extension of dve_ops.OPS).

    EXP16C: w = 1 + y + y^2*(C2 + C1*y)  ~= exp(y) for y = t/16 (6 ALU stages)
    SQ16M:  out = (w^16) * mask          (4 squarings + mult, 5 stages)
    """
    import concourse.dve_ops as dops
    from concourse.dve_ops import DveOp
    from concourse.dve_spec import C1, C2, One, Spec, Src0, Src1, _has_src1, lower, sq
    from concourse.dve_uop import DveOpSpec

    def register(name, spec):
        if name in dops._SUB_OPCODE_FOR_NAME:
            return next(o for o in dops.OPS if o.name == name)
        row = max(dops._SUB_OPCODE_FOR_NAME.values()) + 1
        dops._SUB_OPCODE_FOR_NAME[name] = row
        shas = {}
        for ver in ("v3", "v4"):
            uops = lower(spec, ver=ver)
            shas[ver] = DveOpSpec(
                name=name, opcode=row, uops=uops, rd1_en=_has_src1(spec)
            ).sha(ver)
        op = DveOp(name, spec, subdim=False, uops_sha=shas)
        dops.OPS.append(op)
        dops.CUSTOM_DVE_SPECS[name] = spec
        return op

    exp16c = register(
        "EXP16C",
        Spec(
            body=(sq(Src0) * (Src0 * C1 + C2)) + Src0 + One,
            reference=lambda in0, in1, s0, s1, imm2: (
                1.0 + in0 + in0 * in0 * (imm2 + s1 * in0)
            ),
        ),
    )
    sq16m = register(
        "SQ16M",
        Spec(
            body=sq(sq(sq(sq(Src0)))) * Src1,
            reference=lambda in0, in1, s0, s1, imm2: (
                in0.astype(np.float32) ** 16
            )
            * in1,
        ),
    )
    return exp16c, sq16m


def build():
    if "nc" in _cached:
        return _cached["nc"]
    exp16c, sq16m = _register_dve_ops()
    nc = bacc.Bacc("TRN2", target_bir_lowering=False, debug=False)

    qt_d = nc.dram_tensor("queriesT", [B, D, NQ], BF16, kind="ExternalInput").ap()
    kt_d = nc.dram_tensor("keysT", [B, D, NK], BF16, kind="ExternalInput").ap()
    v_d = nc.dram_tensor(
        "valuesP", [B, 128, KT_TILES, D + 1], BF16, kind="ExternalInput"
    ).ap()
    mb_d = nc.dram_tensor(
        "maskB", [B, 2, 128, NB_T, QH], BF16, kind="ExternalInput"
    ).ap()
    mu_d = nc.dram_tensor(
        "maskU", [B, 2, 128, NU_T, QH], U8, kind="ExternalInput"
    ).ap()
    o_d = nc.dram_tensor("out", [B, 128, QT_TILES, D], BF16, kind="ExternalOutput").ap()

    with tile.TileContext(nc) as tc:
        with (
            tc.tile_pool(name="tr", bufs=3) as trpool,
            tc.tile_pool(name="vbo", bufs=3) as vpool,
            tc.tile_pool(name="maskc", bufs=3) as mpool,
            tc.tile_pool(name="work", bufs=6) as wpool,
            tc.tile_pool(name="wexp", bufs=2) as wepool,
            tc.tile_pool(name="ptslab", bufs=2) as ptpool,
            tc.tile_pool(name="stage", bufs=3) as stpool,
            tc.tile_pool(name="spsum", bufs=3, space="PSUM") as spool,
            tc.tile_pool(name="opsum", bufs=2, space="PSUM") as opool,
        ):
            def q_iter(prev, qc, tail=False):
                """One q-tile of the q-phase for a finished P^T slab."""
                pt, vb, st, b, h, qlo, nq = prev
                qtile = (h * QH + qlo) // 128 + qc
                if tail and qc % 2 == 0:
                    # the s-pool's PSUM banks are free once the last k-phase
                    # is done; alternating rings gives the tail 5 slots
                    o_full = spool.tile([128, QH], F32, tag="s", name="o_tail")
                    o_ps = o_full[:, 0:D + 1]
                else:
                    o_ps = opool.tile([128, D + 1], F32, tag="o")
                for kt in range(KT_TILES):
                    nc.tensor.matmul(
                        o_ps[:],
                        pt[:, kt, qc * 128:(qc + 1) * 128],
                        vb[:, kt, :],
                        start=(kt == 0),
                        stop=(kt == KT_TILES - 1),
                    )
                rd = wpool.tile([128, 1], F32, tag="rd")
                nc.vector.reciprocal(rd[:], o_ps[:, D:D + 1])
                nc.vector.tensor_scalar_mul(st[:, qtile, :], o_ps[:, 0:D], rd[:])
                if qtile % 4 == 3:
                    nc.scalar.dma_start(
                        o_d[b, :, qtile - 3:qtile + 1, :],
                        st[:, qtile - 3:qtile + 1, :],
                    )

            # slabs: (b, h, qlo, qw) — one per (batch, q-half)
            slabs = [(b, h, 0, QH) for b in range(B) for h in range(2)]

            prev = None
            vb = st = kta = ktb = ktc = None
            qth = [None, None]
            for b, h, qlo, qw in slabs:
                if h == 0 and qlo == 0:
                    # per-batch prologue; DMA emission order == consumption
                    # order (serial DMA pipe)
                    kta = trpool.tile([128, 256], BF16, tag="kta")
                    ktb = trpool.tile([128, 768], BF16, tag="ktb")
                    ktc = trpool.tile([128, 1024], BF16, tag="ktc")
                    qth = [
                        trpool.tile([128, QH], BF16, tag=f"qth{hh}", name=f"qth{hh}")
                        for hh in range(2)
                    ]
                    nc.sync.dma_start(kta[:], kt_d[b, :, 0:256])
                    nc.sync.dma_start(qth[0][:], qt_d[b, :, 0:QH])
                    vb = vpool.tile([128, KT_TILES, D + 1], BF16, tag="vb")
                    st = stpool.tile([128, QT_TILES, D], BF16, tag="st")

                qsl = slice(qlo, qlo + qw)
                mba = mpool.tile([128, 2, QH], BF16, tag="mba")
                mbb = mpool.tile([128, NB_T - 2, QH], BF16, tag="mbb")
                mua = mpool.tile([128, 2, QH], U8, tag="mua")
                mub = mpool.tile([128, NU_T - 2, QH], U8, tag="mub")
                nc.sync.dma_start(mba[:, :, 0:qw], mb_d[b, h, :, 0:2, qsl])
                nc.sync.dma_start(mua[:, :, 0:qw], mu_d[b, h, :, 0:2, qsl])
                if h == 0 and qlo == 0:
                    nc.sync.dma_start(ktb[:], kt_d[b, :, 256:1024])
                nc.sync.dma_start(mbb[:, :, 0:qw], mb_d[b, h, :, 2:NB_T, qsl])
                if h == 0 and qlo == 0:
                    nc.sync.dma_start(ktc[:], kt_d[b, :, 1024:NK])
                    nc.sync.dma_start(qth[1][:], qt_d[b, :, QH:NQ])
                    nc.sync.dma_start(vb[:], v_d[b])
                nc.sync.dma_start(mub[:, :, 0:qw], mu_d[b, h, :, 2:NU_T, qsl])

                def mask_bf(kt):
                    i = BF_IDX[kt]
                    return mba[:, i, 0:qw] if i < 2 else mbb[:, i - 2, 0:qw]

                def mask_u8(kt, three_d=False):
                    i = U8_IDX[kt]
                    t_ = mua if i < 2 else mub
                    i_ = i if i < 2 else i - 2
                    if three_d:
                        return t_[:, i_:i_ + 1, 0:qw]
                    return t_[:, i_, 0:qw]

                # interleave slots for the previous slab's q-phase: pack into
                # the back half so a late pt slab can't stall the PE queue
                if prev is not None:
                    p_nq = prev[6]
                    if p_nq == 8:
                        qslot = {7 + j: j for j in range(8)}
                    else:
                        qslot = {8 + 2 * j: j for j in range(p_nq)}
                else:
                    qslot = {}

                pt = ptpool.tile([128, KT_TILES, QH], BF16, tag="pt")
                for kt in range(KT_TILES):
                    s_ps = spool.tile([128, QH], F32, tag="s")
                    for c in range(qw // 512):
                        nc.tensor.matmul(
                            s_ps[:, c * 512:(c + 1) * 512],
                            kta[:, kt * 128:(kt + 1) * 128]
                            if kt < 2
                            else (
                                ktb[:, (kt - 2) * 128:(kt - 1) * 128]
                                if kt < 8
                                else ktc[:, (kt - 8) * 128:(kt - 7) * 128]
                            ),
                            qth[h][:, qlo + c * 512:qlo + (c + 1) * 512],
                            start=True,
                            stop=True,
                        )
                    if kt in KT_CUSTOM:
                        w_sb = wepool.tile([128, QH], F32, tag="w")
                        nc.vector._custom_dve(
                            exp16c,
                            out=w_sb[:, 0:qw],
                            in0=s_ps[:, 0:qw],
                            s1=A3,
                            imm2=B2,
                        )
                        nc.vector._custom_dve(
                            sq16m,
                            out=pt[:, kt, 0:qw],
                            in0=w_sb[:, 0:qw],
                            in1=mask_u8(kt, three_d=True),
                        )
                    else:
                        e_sb = wpool.tile([128, QH], BF16, tag="e")
                        nc.scalar.activation(
                            e_sb[:, 0:qw],
                            s_ps[:, 0:qw],
                            mybir.ActivationFunctionType.Exp,
                            scale=16.0,
                        )
                        # final slab: route the trailing Pool mults (kt 11/15)
                        # to DVE so a lagging Pool queue can't gate the drain
                        # tail's first mm2 (and reset the PE clock ramp)
                        last_slab = (b, h, qlo, qw) == slabs[-1]
                        if kt in KT_DVE or (last_slab and kt in (11, 15)):
                            nc.vector.tensor_tensor(
                                out=pt[:, kt, 0:qw],
                                in0=e_sb[:, 0:qw],
                                in1=mask_bf(kt) if kt in KT_DVE else mask_u8(kt),
                                op=mybir.AluOpType.mult,
                            )
                        else:
                            nc.gpsimd.tensor_tensor(
                                out=pt[:, kt, 0:qw],
                                in0=e_sb[:, 0:qw],
                                in1=mask_u8(kt),
                                op=mybir.AluOpType.mult,
                            )
                    if kt in qslot:
                        q_iter(prev, qslot[kt])
                prev = (pt, vb, st, b, h, qlo, qw // 128)

            for qc in range(prev[6]):
                q_iter(prev, qc, tail=True)

    nc.compile()
    _cached["nc"] = nc
    return nc


def kernel(queries, keys, values, mask, _trace=False, **kw):
    queries = np.asarray(queries, dtype=np.float32)
    keys = np.asarray(keys, dtype=np.float32)
    values = np.asarray(values, dtype=np.float32)
    mask = np.asarray(mask, dtype=np.float32)
    nc = build()
    in_maps = []
    for c in range(N_CORES):
        sl = slice(c * B, (c + 1) * B)
        # [V | 1]: ones column so P @ [V|1] emits the denominator
        vp = np.ones((B, KT_TILES, 128, D + 1), dtype=np.float32)
        vp[:, :, :, :D] = values[sl].reshape(B, KT_TILES, 128, D)
        # mask^T packed per (b, h): [B, 2, 128, KT, QH]; then split per
        # k-tile into bf16*255 and u8 copies (one common 255 scale — it
        # cancels in the renormalization)
        mt = (
            mask[sl]
            .transpose(0, 2, 1)  # [B, k, q]
            .reshape(B, KT_TILES, 128, 2, QH)
            .transpose(0, 3, 2, 1, 4)  # [B, 2, 128p, KT, QH]
        )
        mb = (mt[:, :, :, KT_DVE, :] * 255.0).astype(ml_dtypes.bfloat16)
        mu = np.rint(mt[:, :, :, KT_U8, :] * 255.0).astype(np.uint8)
        in_maps.append(
            {
                "queriesT": np.ascontiguousarray(
                    queries[sl].transpose(0, 2, 1) * (SCALE / 16.0)
                ).astype(ml_dtypes.bfloat16),
                "keysT": np.ascontiguousarray(
                    keys[sl].transpose(0, 2, 1)
                ).astype(ml_dtypes.bfloat16),
                "valuesP": np.ascontiguousarray(
                    vp.transpose(0, 2, 1, 3)
                ).astype(ml_dtypes.bfloat16),
                "maskB": np.ascontiguousarray(mb),
                "maskU": np.ascontiguousarray(mu),
            }
        )
    res = run_bass_kernel_spmd(
        nc, in_maps, core_ids=list(range(N_CORES)), trace=_trace
    )
    out = np.concatenate(
        [
            res.results[c]["out"]
            .astype(np.float32)
            .transpose(0, 2, 1, 3)
            .reshape(B, NQ, D)
            for c in range(N_CORES)
        ],
        axis=0,
    )
    if _trace:
        return out, res
    return out


# revision 54
# speedup vs baseline: 1.0371x; 1.0344x over previous
"""Trainium2 Bass kernel for masked-softmax attention (sparse_attention).

reference:
    S = Q @ K^T / sqrt(128)            # [N, nq, nk]
    A = softmax(S, axis=2)
    A = A * mask;  A = A / (sum_k A + 1e-6)
    O = A @ V

Device identity (softmax normalizer and any constant mask scale cancel in
the renormalization):
    E = exp(S); P = E * (mask*255)
    O[q, :] = (P @ V)[q, :] / sum_k P[q, k]

Sharding: N=32 batch-heads split across 8 NeuronCores, 4 per core; no
cross-core communication. Host staging: Q/K transposed to [d, n] bf16 with
Q pre-scaled by 1/(sqrt(d)*16) (so PSUM scores are t/16 — the activation
rescales by 16 and the poly-exp path consumes t/16 directly), V tiled with
a ones column appended (P @ [V|1] yields the renorm denominator inside
mm2), mask transposed to [k, q] and split per k-tile between bf16*255 and
u8 copies.

The exponential is the scarce resource (ACT does 1 elem/cycle @1.2GHz =
109us/core just for exp, vs PE's 110us of matmul): per (batch, q-half)
slab the 16 k-tiles are split across engines so every engine lands at
~110us/core:
  7 tiles: ACT exp -> DVE mult (bf16 mask, DVE 2x mode)
  6 tiles: ACT exp -> Pool (gpsimd) mult (u8 mask)
  3 tiles: custom-DVE exp: cubic poly of t/16, then (.)^16 * mask fused
           (two 1-pass custom DVE ops, skips ACT entirely)

Per-core pipeline, per (batch b, q-half h of 1024):
  k-phase, per k-tile kt: mm1 (PE) -> exp -> mask-mult -> P^T slab
  q-phase (interleaved into the next k-phase at kt 7..14), per q-tile:
    mm2 (PE): O|denom = sum_kt PT[kt].T @ [V_kt|1] -> PSUM
    recip+scale (DVE): st = O * (1/denom)
  store st -> out in 4-q-tile chunks on the ACT HWDGE ring.
"""
import sys

sys.path.insert(0, "/opt/trn_rl_repo")

import ml_dtypes
import numpy as np

from concourse import bacc, mybir, tile
from concourse.bass_utils import run_bass_kernel_spmd

N, NQ, NK, D = 32, 2048, 2048, 128
N_CORES = 8
B = N // N_CORES          # batches per core
QT_TILES = NQ // 128      # q tiles per batch
KT_TILES = NK // 128      # k tiles per batch
QH = NQ // 2              # q-half width
SCALE = float(1.0 / np.sqrt(D))

# cubic minimax fit of e^y on [-0.5375, 0.5375] with p(y)=1+y+y^2*(B2+A3*y)
A3 = 0.17059872676988808
B2 = 0.5101347134234719

# per-slab k-tile engine assignment
KT_CUSTOM = (3, 9, 13)                      # custom-DVE exp^16 path (u8 mask)
KT_POOL = (1, 5, 7, 8, 11, 15)              # ACT exp -> Pool mult (u8 mask)
KT_DVE = (0, 2, 4, 6, 10, 12, 14)           # ACT exp -> DVE mult (bf16 mask)
KT_U8 = tuple(sorted(KT_CUSTOM + KT_POOL))  # u8-mask tiles, in kt order
NB_T = len(KT_DVE)
NU_T = len(KT_U8)
U8_IDX = {kt: i for i, kt in enumerate(KT_U8)}
BF_IDX = {kt: i for i, kt in enumerate(KT_DVE)}

F32 = mybir.dt.float32
BF16 = mybir.dt.bfloat16
U8 = mybir.dt.uint8

_cached = {}


def _register_dve_ops():
    """Register the two custom DVE ops (runtime extension of dve_ops.OPS).

    EXP16C: w = 1 + y + y^2*(C2 + C1*y)  ~= exp(y) for y = t/16 (6 ALU stages)
    SQ16M:  out = (w^16) * mask          (4 squarings + mult, 5 stages)
    """
    import concourse.dve_ops as dops
    from concourse.dve_ops import DveOp
    from concourse.dve_spec import C1, C2, One, Spec, Src0, Src1, _has_src1, lower, sq
    from concourse.dve_uop import DveOpSpec

    def register(name, spec):
        if name in dops._SUB_OPCODE_FOR_NAME:
            return next(o for o in dops.OPS if o.name == name)
        row = max(dops._SUB_OPCODE_FOR_NAME.values()) + 1
        dops._SUB_OPCODE_FOR_NAME[name] = row
        shas = {}
        for ver in ("v3", "v4"):
            uops = lower(spec, ver=ver)
            shas[ver] = DveOpSpec(
                name=name, opcode=row, uops=uops, rd1_en=_has_src1(spec)
            ).sha(ver)
        op = DveOp(name, spec, subdim=False, uops_sha=shas)
        dops.OPS.append(op)
        dops.CUSTOM_DVE_SPECS[name] = spec
        return op

    exp16c = register(
        "EXP16C",
        Spec(
            body=(sq(Src0) * (Src0 * C1 + C2)) + Src0 + One,
            reference=lambda in0, in1, s0, s1, imm2: (
                1.0 + in0 + in0 * in0 * (imm2 + s1 * in0)
            ),
        ),
    )
    sq16m = register(
        "SQ16M",
        Spec(
            body=sq(sq(sq(sq(Src0)))) * Src1,
            reference=lambda in0, in1, s0, s1, imm2: (
                in0.astype(np.float32) ** 16
            )
            * in1,
        ),
    )
    return exp16c, sq16m


def build():
    if "nc" in _cached:
        return _cached["nc"]
    exp16c, sq16m = _register_dve_ops()
    nc = bacc.Bacc("TRN2", target_bir_lowering=False, debug=False)

    qt_d = nc.dram_tensor("queriesT", [B, D, NQ], BF16, kind="ExternalInput").ap()
    kt_d = nc.dram_tensor("keysT", [B, D, NK], BF16, kind="ExternalInput").ap()
    v_d = nc.dram_tensor(
        "valuesP", [B, 128, KT_TILES, D + 1], BF16, kind="ExternalInput"
    ).ap()
    mb_d = nc.dram_tensor(
        "maskB", [B, 2, 128, NB_T, QH], BF16, kind="ExternalInput"
    ).ap()
    mu_d = nc.dram_tensor(
        "maskU", [B, 2, 128, NU_T, QH], U8, kind="ExternalInput"
    ).ap()
    o_d = nc.dram_tensor("out", [B, 128, QT_TILES, D], BF16, kind="ExternalOutput").ap()

    with tile.TileContext(nc) as tc:
        with (
            tc.tile_pool(name="tr", bufs=3) as trpool,
            tc.tile_pool(name="vbo", bufs=3) as vpool,
            tc.tile_pool(name="maskc", bufs=3) as mpool,
            tc.tile_pool(name="work", bufs=6) as wpool,
            tc.tile_pool(name="wexp", bufs=2) as wepool,
            tc.tile_pool(name="ptslab", bufs=2) as ptpool,
            tc.tile_pool(name="stage", bufs=3) as stpool,
            tc.tile_pool(name="spsum", bufs=3, space="PSUM") as spool,
            tc.tile_pool(name="opsum", bufs=2, space="PSUM") as opool,
        ):
            def q_iter(prev, qc, tail=False):
                """One q-tile of the q-phase for a finished P^T slab."""
                pt, vb, st, b, h, qlo, nq = prev
                qtile = (h * QH + qlo) // 128 + qc
                if tail and qc % 2 == 0:
                    # the s-pool's PSUM banks are free once the last k-phase
                    # is done; alternating rings gives the tail 5 slots
                    o_full = spool.tile([128, QH], F32, tag="s", name="o_tail")
                    o_ps = o_full[:, 0:D + 1]
                else:
                    o_ps = opool.tile([128, D + 1], F32, tag="o")
                for kt in range(KT_TILES):
                    nc.tensor.matmul(
                        o_ps[:],
                        pt[:, kt, qc * 128:(qc + 1) * 128],
                        vb[:, kt, :],
                        start=(kt == 0),
                        stop=(kt == KT_TILES - 1),
                    )
                rd = wpool.tile([128, 1], F32, tag="rd")
                nc.vector.reciprocal(rd[:], o_ps[:, D:D + 1])
                nc.vector.tensor_scalar_mul(st[:, qtile, :], o_ps[:, 0:D], rd[:])
                if qtile % 4 == 3:
                    nc.scalar.dma_start(
                        o_d[b, :, qtile - 3:qtile + 1, :],
                        st[:, qtile - 3:qtile + 1, :],
                    )

            # slabs: (b, h, qlo, qw) — one per (batch, q-half)
            slabs = [(b, h, 0, QH) for b in range(B) for h in range(2)]

            prev = None
            vb = st = kta = ktb = ktc = None
            qth = [None, None]
            for b, h, qlo, qw in slabs:
                if h == 0 and qlo == 0:
                    # per-batch prologue; DMA emission order == consumption
                    # order (serial DMA pipe)
                    kta = trpool.tile([128, 256], BF16, tag="kta")
                    ktb = trpool.tile([128, 768], BF16, tag="ktb")
                    ktc = trpool.tile([128, 1024], BF16, tag="ktc")
                    qth = [
                        trpool.tile([128, QH], BF16, tag=f"qth{hh}", name=f"qth{hh}")
                        for hh in range(2)
                    ]
                    nc.sync.dma_start(kta[:], kt_d[b, :, 0:256])
                    nc.sync.dma_start(qth[0][:], qt_d[b, :, 0:QH])
                    vb = vpool.tile([128, KT_TILES, D + 1], BF16, tag="vb")
                    st = stpool.tile([128, QT_TILES, D], BF16, tag="st")

                qsl = slice(qlo, qlo + qw)
                mba = mpool.tile([128, 2, QH], BF16, tag="mba")
                mbb = mpool.tile([128, 2, QH], BF16, tag="mbb")
                mbc = mpool.tile([128, NB_T - 4, QH], BF16, tag="mbc")
                mua = mpool.tile([128, 2, QH], U8, tag="mua")
                mub = mpool.tile([128, 3, QH], U8, tag="mub")
                muc = mpool.tile([128, NU_T - 5, QH], U8, tag="muc")
                nc.sync.dma_start(mba[:, :, 0:qw], mb_d[b, h, :, 0:2, qsl])
                nc.sync.dma_start(mua[:, :, 0:qw], mu_d[b, h, :, 0:2, qsl])
                if h == 0 and qlo == 0:
                    nc.sync.dma_start(ktb[:], kt_d[b, :, 256:1024])
                nc.sync.dma_start(mbb[:, :, 0:qw], mb_d[b, h, :, 2:4, qsl])
                nc.sync.dma_start(mub[:, :, 0:qw], mu_d[b, h, :, 2:5, qsl])
                if h == 0 and qlo == 0:
                    nc.sync.dma_start(ktc[:], kt_d[b, :, 1024:NK])
                    nc.sync.dma_start(qth[1][:], qt_d[b, :, QH:NQ])
                nc.sync.dma_start(mbc[:, :, 0:qw], mb_d[b, h, :, 4:NB_T, qsl])
                if h == 0 and qlo == 0:
                    nc.sync.dma_start(vb[:], v_d[b])
                nc.sync.dma_start(muc[:, :, 0:qw], mu_d[b, h, :, 5:NU_T, qsl])

                def mask_bf(kt):
                    i = BF_IDX[kt]
                    t_, i_ = (
                        (mba, i) if i < 2 else (mbb, i - 2) if i < 4 else (mbc, i - 4)
                    )
                    return t_[:, i_, 0:qw]

                def mask_u8(kt, three_d=False):
                    i = U8_IDX[kt]
                    t_, i_ = (
                        (mua, i) if i < 2 else (mub, i - 2) if i < 5 else (muc, i - 5)
                    )
                    if three_d:
                        return t_[:, i_:i_ + 1, 0:qw]
                    return t_[:, i_, 0:qw]

                # interleave slots for the previous slab's q-phase: pack into
                # the back half so a late pt slab can't stall the PE queue
                if prev is not None:
                    p_nq = prev[6]
                    if p_nq == 8:
                        qslot = {7 + j: j for j in range(8)}
                    else:
                        qslot = {8 + 2 * j: j for j in range(p_nq)}
                else:
                    qslot = {}

                pt = ptpool.tile([128, KT_TILES, QH], BF16, tag="pt")
                for kt in range(KT_TILES):
                    s_ps = spool.tile([128, QH], F32, tag="s")
                    for c in range(qw // 512):
                        nc.tensor.matmul(
                            s_ps[:, c * 512:(c + 1) * 512],
                            kta[:, kt * 128:(kt + 1) * 128]
                            if kt < 2
                            else (
                                ktb[:, (kt - 2) * 128:(kt - 1) * 128]
                                if kt < 8
                                else ktc[:, (kt - 8) * 128:(kt - 7) * 128]
                            ),
                            qth[h][:, qlo + c * 512:qlo + (c + 1) * 512],
                            start=True,
                            stop=True,
                        )
                    if kt in KT_CUSTOM:
                        w_sb = wepool.tile([128, QH], F32, tag="w")
                        nc.vector._custom_dve(
                            exp16c,
                            out=w_sb[:, 0:qw],
                            in0=s_ps[:, 0:qw],
                            s1=A3,
                            imm2=B2,
                        )
                        nc.vector._custom_dve(
                            sq16m,
                            out=pt[:, kt, 0:qw],
                            in0=w_sb[:, 0:qw],
                            in1=mask_u8(kt, three_d=True),
                        )
                    else:
                        e_sb = wpool.tile([128, QH], BF16, tag="e")
                        nc.scalar.activation(
                            e_sb[:, 0:qw],
                            s_ps[:, 0:qw],
                            mybir.ActivationFunctionType.Exp,
                            scale=16.0,
                        )
                        # final slab: route the trailing Pool mults (kt 11/15)
                        # to DVE so a lagging Pool queue can't gate the drain
                        # tail's first mm2 (and reset the PE clock ramp)
                        last_slab = (b, h, qlo, qw) == slabs[-1]
                        if kt in KT_DVE or (last_slab and kt in (11, 15)):
                            nc.vector.tensor_tensor(
                                out=pt[:, kt, 0:qw],
                                in0=e_sb[:, 0:qw],
                                in1=mask_bf(kt) if kt in KT_DVE else mask_u8(kt),
                                op=mybir.AluOpType.mult,
                            )
                        else:
                            nc.gpsimd.tensor_tensor(
                                out=pt[:, kt, 0:qw],
                                in0=e_sb[:, 0:qw],
                                in1=mask_u8(kt),
                                op=mybir.AluOpType.mult,
                            )
                    if kt in qslot:
                        q_iter(prev, qslot[kt])
                prev = (pt, vb, st, b, h, qlo, qw // 128)

            for qc in range(prev[6]):
                q_iter(prev, qc, tail=True)

    nc.compile()
    _cached["nc"] = nc
    return nc


def kernel(queries, keys, values, mask, _trace=False, **kw):
    queries = np.asarray(queries, dtype=np.float32)
    keys = np.asarray(keys, dtype=np.float32)
    values = np.asarray(values, dtype=np.float32)
    mask = np.asarray(mask, dtype=np.float32)
    nc = build()
    in_maps = []
    for c in range(N_CORES):
        sl = slice(c * B, (c + 1) * B)
        # [V | 1]: ones column so P @ [V|1] emits the denominator
        vp = np.ones((B, KT_TILES, 128, D + 1), dtype=np.float32)
        vp[:, :, :, :D] = values[sl].reshape(B, KT_TILES, 128, D)
        # mask^T packed per (b, h): [B, 2, 128, KT, QH]; then split per
        # k-tile into bf16*255 and u8 copies (one common 255 scale — it
        # cancels in the renormalization)
        mt = (
            mask[sl]
            .transpose(0, 2, 1)  # [B, k, q]
            .reshape(B, KT_TILES, 128, 2, QH)
            .transpose(0, 3, 2, 1, 4)  # [B, 2, 128p, KT, QH]
        )
        mb = (mt[:, :, :, KT_DVE, :] * 255.0).astype(ml_dtypes.bfloat16)
        mu = np.rint(mt[:, :, :, KT_U8, :] * 255.0).astype(np.uint8)
        in_maps.append(
            {
                "queriesT": np.ascontiguousarray(
                    queries[sl].transpose(0, 2, 1) * (SCALE / 16.0)
                ).astype(ml_dtypes.bfloat16),
                "keysT": np.ascontiguousarray(
                    keys[sl].transpose(0, 2, 1)
                ).astype(ml_dtypes.bfloat16),
                "valuesP": np.ascontiguousarray(
                    vp.transpose(0, 2, 1, 3)
                ).astype(ml_dtypes.bfloat16),
                "maskB": np.ascontiguousarray(mb),
                "maskU": np.ascontiguousarray(mu),
            }
        )
    res = run_bass_kernel_spmd(
        nc, in_maps, core_ids=list(range(N_CORES)), trace=_trace
    )
    out = np.concatenate(
        [
            res.results[c]["out"]
            .astype(np.float32)
            .transpose(0, 2, 1, 3)
            .reshape(B, NQ, D)
            for c in range(N_CORES)
        ],
        axis=0,
    )
    if _trace:
        return out, res
    return out


# revision 59
# speedup vs baseline: 1.0373x; 1.0002x over previous
"""Trainium2 Bass kernel for masked-softmax attention (sparse_attention).

reference:
    S = Q @ K^T / sqrt(128)            # [N, nq, nk]
    A = softmax(S, axis=2)
    A = A * mask;  A = A / (sum_k A + 1e-6)
    O = A @ V

Device identity (softmax normalizer and any constant mask scale cancel in
the renormalization):
    E = exp(S); P = E * (mask*255)
    O[q, :] = (P @ V)[q, :] / sum_k P[q, k]

Sharding: N=32 batch-heads split across 8 NeuronCores, 4 per core; no
cross-core communication. Host staging: Q/K transposed to [d, n] bf16 with
Q pre-scaled by 1/(sqrt(d)*16) (so PSUM scores are t/16 — the activation
rescales by 16 and the poly-exp path consumes t/16 directly), V tiled with
a ones column appended (P @ [V|1] yields the renorm denominator inside
mm2), mask transposed to [k, q] and split per k-tile between bf16*255 and
u8 copies.

The exponential is the scarce resource (ACT does 1 elem/cycle @1.2GHz =
109us/core just for exp, vs PE's 110us of matmul): per (batch, q-half)
slab the 16 k-tiles are split across engines so every engine lands at
~110us/core:
  7 tiles: ACT exp -> DVE mult (bf16 mask, DVE 2x mode)
  6 tiles: ACT exp -> Pool (gpsimd) mult (u8 mask)
  3 tiles: custom-DVE exp: cubic poly of t/16, then (.)^16 * mask fused
           (two 1-pass custom DVE ops, skips ACT entirely)

Per-core pipeline, per (batch b, q-half h of 1024):
  k-phase, per k-tile kt: mm1 (PE) -> exp -> mask-mult -> P^T slab
  q-phase (interleaved into the next k-phase at kt 7..14), per q-tile:
    mm2 (PE): O|denom = sum_kt PT[kt].T @ [V_kt|1] -> PSUM
    recip+scale (DVE): st = O * (1/denom)
  store st -> out in 4-q-tile chunks on the ACT HWDGE ring.
"""
import sys

sys.path.insert(0, "/opt/trn_rl_repo")

import ml_dtypes
import numpy as np

from concourse import bacc, mybir, tile
from concourse.bass_utils import run_bass_kernel_spmd

N, NQ, NK, D = 32, 2048, 2048, 128
N_CORES = 8
B = N // N_CORES          # batches per core
QT_TILES = NQ // 128      # q tiles per batch
KT_TILES = NK // 128      # k tiles per batch
QH = NQ // 2              # q-half width
SCALE = float(1.0 / np.sqrt(D))

# cubic minimax fit of e^y on [-0.5375, 0.5375] with p(y)=1+y+y^2*(B2+A3*y)
A3 = 0.17059872676988808
B2 = 0.5101347134234719

# per-slab k-tile engine assignment
KT_CUSTOM = (3, 9, 13)                      # custom-DVE exp^16 path (u8 mask)
KT_POOL = (1, 5, 7, 8, 11, 15)              # ACT exp -> Pool mult (u8 mask)
KT_DVE = (0, 2, 4, 6, 10, 12, 14)           # ACT exp -> DVE mult (bf16 mask)
KT_U8 = tuple(sorted(KT_CUSTOM + KT_POOL))  # u8-mask tiles, in kt order
NB_T = len(KT_DVE)
NU_T = len(KT_U8)
U8_IDX = {kt: i for i, kt in enumerate(KT_U8)}
BF_IDX = {kt: i for i, kt in enumerate(KT_DVE)}

F32 = mybir.dt.float32
BF16 = mybir.dt.bfloat16
U8 = mybir.dt.uint8

_cached = {}


def _register_dve_ops():
    """Register the two custom DVE ops (runtime extension of dve_ops.OPS).

    EXP16C: w = 1 + y + y^2*(C2 + C1*y)  ~= exp(y) for y = t/16 (6 ALU stages)
    SQ16M:  out = (w^16) * mask          (4 squarings + mult, 5 stages)
    """
    import concourse.dve_ops as dops
    from concourse.dve_ops import DveOp
    from concourse.dve_spec import C1, C2, One, Spec, Src0, Src1, _has_src1, lower, sq
    from concourse.dve_uop import DveOpSpec

    def register(name, spec):
        if name in dops._SUB_OPCODE_FOR_NAME:
            return next(o for o in dops.OPS if o.name == name)
        row = max(dops._SUB_OPCODE_FOR_NAME.values()) + 1
        dops._SUB_OPCODE_FOR_NAME[name] = row
        shas = {}
        for ver in ("v3", "v4"):
            uops = lower(spec, ver=ver)
            shas[ver] = DveOpSpec(
                name=name, opcode=row, uops=uops, rd1_en=_has_src1(spec)
            ).sha(ver)
        op = DveOp(name, spec, subdim=False, uops_sha=shas)
        dops.OPS.append(op)
        dops.CUSTOM_DVE_SPECS[name] = spec
        return op

    exp16c = register(
        "EXP16C",
        Spec(
            body=(sq(Src0) * (Src0 * C1 + C2)) + Src0 + One,
            reference=lambda in0, in1, s0, s1, imm2: (
                1.0 + in0 + in0 * in0 * (imm2 + s1 * in0)
            ),
        ),
    )
    sq16m = register(
        "SQ16M",
        Spec(
            body=sq(sq(sq(sq(Src0)))) * Src1,
            reference=lambda in0, in1, s0, s1, imm2: (
                in0.astype(np.float32) ** 16
            )
            * in1,
        ),
    )
    return exp16c, sq16m


def build():
    if "nc" in _cached:
        return _cached["nc"]
    exp16c, sq16m = _register_dve_ops()
    nc = bacc.Bacc("TRN2", target_bir_lowering=False, debug=False)

    qt_d = nc.dram_tensor("queriesT", [B, D, NQ], BF16, kind="ExternalInput").ap()
    kt_d = nc.dram_tensor("keysT", [B, D, NK], BF16, kind="ExternalInput").ap()
    v_d = nc.dram_tensor(
        "valuesP", [B, 128, KT_TILES, D + 1], BF16, kind="ExternalInput"
    ).ap()
    mb_d = nc.dram_tensor(
        "maskB", [B, 2, 128, NB_T, QH], BF16, kind="ExternalInput"
    ).ap()
    mu_d = nc.dram_tensor(
        "maskU", [B, 2, 128, NU_T, QH], U8, kind="ExternalInput"
    ).ap()
    o_d = nc.dram_tensor("out", [B, 128, QT_TILES, D], BF16, kind="ExternalOutput").ap()

    with tile.TileContext(nc) as tc:
        with (
            tc.tile_pool(name="tr", bufs=3) as trpool,
            tc.tile_pool(name="vbo", bufs=3) as vpool,
            tc.tile_pool(name="maskc", bufs=3) as mpool,
            tc.tile_pool(name="work", bufs=6) as wpool,
            tc.tile_pool(name="wexp", bufs=2) as wepool,
            tc.tile_pool(name="ptslab", bufs=2) as ptpool,
            tc.tile_pool(name="stage", bufs=3) as stpool,
            tc.tile_pool(name="spsum", bufs=3, space="PSUM") as spool,
            tc.tile_pool(name="opsum", bufs=2, space="PSUM") as opool,
        ):
            def q_iter(prev, qc, tail=False):
                """One q-tile of the q-phase for a finished P^T slab."""
                pt, vb, st, b, h, qlo, nq = prev
                qtile = (h * QH + qlo) // 128 + qc
                if tail and qc % 2 == 0:
                    # the s-pool's PSUM banks are free once the last k-phase
                    # is done; alternating rings gives the tail 5 slots
                    o_full = spool.tile([128, QH], F32, tag="s", name="o_tail")
                    o_ps = o_full[:, 0:D + 1]
                else:
                    o_ps = opool.tile([128, D + 1], F32, tag="o")
                for kt in range(KT_TILES):
                    nc.tensor.matmul(
                        o_ps[:],
                        pt[:, kt, qc * 128:(qc + 1) * 128],
                        vb[:, kt, :],
                        start=(kt == 0),
                        stop=(kt == KT_TILES - 1),
                    )
                rd = wpool.tile([128, 1], F32, tag="rd")
                nc.vector.reciprocal(rd[:], o_ps[:, D:D + 1])
                nc.vector.tensor_scalar_mul(st[:, qtile, :], o_ps[:, 0:D], rd[:])
                last_b = b == B - 1
                if qtile % 4 == 3 and not (last_b and qtile == 15):
                    nc.scalar.dma_start(
                        o_d[b, :, qtile - 3:qtile + 1, :],
                        st[:, qtile - 3:qtile + 1, :],
                    )
                elif last_b and qtile in (13, 15):
                    lo, hi = (12, 14) if qtile == 13 else (14, 16)
                    nc.scalar.dma_start(
                        o_d[b, :, lo:hi, :], st[:, lo:hi, :]
                    )

            # slabs: (b, h, qlo, qw) — one per (batch, q-half)
            slabs = [(b, h, 0, QH) for b in range(B) for h in range(2)]

            prev = None
            vb = st = kta = ktb = ktc = None
            qth = [None, None]
            for b, h, qlo, qw in slabs:
                if h == 0 and qlo == 0:
                    # per-batch prologue; DMA emission order == consumption
                    # order (serial DMA pipe)
                    kta = trpool.tile([128, 256], BF16, tag="kta")
                    ktb = trpool.tile([128, 768], BF16, tag="ktb")
                    ktc = trpool.tile([128, 1024], BF16, tag="ktc")
                    qth = [
                        trpool.tile([128, QH], BF16, tag=f"qth{hh}", name=f"qth{hh}")
                        for hh in range(2)
                    ]
                    nc.sync.dma_start(kta[:], kt_d[b, :, 0:256])
                    nc.sync.dma_start(qth[0][:], qt_d[b, :, 0:QH])
                    vb = vpool.tile([128, KT_TILES, D + 1], BF16, tag="vb")
                    st = stpool.tile([128, QT_TILES, D], BF16, tag="st")

                qsl = slice(qlo, qlo + qw)
                mba = mpool.tile([128, 2, QH], BF16, tag="mba")
                mbb = mpool.tile([128, 2, QH], BF16, tag="mbb")
                mbc = mpool.tile([128, NB_T - 4, QH], BF16, tag="mbc")
                mua = mpool.tile([128, 2, QH], U8, tag="mua")
                mub = mpool.tile([128, 3, QH], U8, tag="mub")
                muc = mpool.tile([128, NU_T - 5, QH], U8, tag="muc")
                nc.sync.dma_start(mba[:, :, 0:qw], mb_d[b, h, :, 0:2, qsl])
                nc.sync.dma_start(mua[:, :, 0:qw], mu_d[b, h, :, 0:2, qsl])
                if h == 0 and qlo == 0:
                    nc.sync.dma_start(ktb[:], kt_d[b, :, 256:1024])
                nc.sync.dma_start(mbb[:, :, 0:qw], mb_d[b, h, :, 2:4, qsl])
                nc.sync.dma_start(mub[:, :, 0:qw], mu_d[b, h, :, 2:5, qsl])
                if h == 0 and qlo == 0:
                    nc.sync.dma_start(ktc[:], kt_d[b, :, 1024:NK])
                    nc.sync.dma_start(qth[1][:], qt_d[b, :, QH:NQ])
                nc.sync.dma_start(mbc[:, :, 0:qw], mb_d[b, h, :, 4:NB_T, qsl])
                if h == 0 and qlo == 0:
                    nc.sync.dma_start(vb[:], v_d[b])
                nc.sync.dma_start(muc[:, :, 0:qw], mu_d[b, h, :, 5:NU_T, qsl])

                def mask_bf(kt):
                    i = BF_IDX[kt]
                    t_, i_ = (
                        (mba, i) if i < 2 else (mbb, i - 2) if i < 4 else (mbc, i - 4)
                    )
                    return t_[:, i_, 0:qw]

                def mask_u8(kt, three_d=False):
                    i = U8_IDX[kt]
                    t_, i_ = (
                        (mua, i) if i < 2 else (mub, i - 2) if i < 5 else (muc, i - 5)
                    )
                    if three_d:
                        return t_[:, i_:i_ + 1, 0:qw]
                    return t_[:, i_, 0:qw]

                # interleave slots for the previous slab's q-phase: pack into
                # the back half so a late pt slab can't stall the PE queue
                if prev is not None:
                    p_nq = prev[6]
                    if p_nq == 8:
                        qslot = {7 + j: j for j in range(8)}
                    else:
                        qslot = {8 + 2 * j: j for j in range(p_nq)}
                else:
                    qslot = {}

                pt = ptpool.tile([128, KT_TILES, QH], BF16, tag="pt")
                for kt in range(KT_TILES):
                    s_ps = spool.tile([128, QH], F32, tag="s")
                    for c in range(qw // 512):
                        nc.tensor.matmul(
                            s_ps[:, c * 512:(c + 1) * 512],
                            kta[:, kt * 128:(kt + 1) * 128]
                            if kt < 2
                            else (
                                ktb[:, (kt - 2) * 128:(kt - 1) * 128]
                                if kt < 8
                                else ktc[:, (kt - 8) * 128:(kt - 7) * 128]
                            ),
                            qth[h][:, qlo + c * 512:qlo + (c + 1) * 512],
                            start=True,
                            stop=True,
                        )
                    if kt in KT_CUSTOM:
                        w_sb = wepool.tile([128, QH], F32, tag="w")
                        nc.vector._custom_dve(
                            exp16c,
                            out=w_sb[:, 0:qw],
                            in0=s_ps[:, 0:qw],
                            s1=A3,
                            imm2=B2,
                        )
                        nc.vector._custom_dve(
                            sq16m,
                            out=pt[:, kt, 0:qw],
                            in0=w_sb[:, 0:qw],
                            in1=mask_u8(kt, three_d=True),
                        )
                    else:
                        e_sb = wpool.tile([128, QH], BF16, tag="e")
                        nc.scalar.activation(
                            e_sb[:, 0:qw],
                            s_ps[:, 0:qw],
                            mybir.ActivationFunctionType.Exp,
                            scale=16.0,
                        )
                        # final slab: route the trailing Pool mults (kt 11/15)
                        # to DVE so a lagging Pool queue can't gate the drain
                        # tail's first mm2 (and reset the PE clock ramp)
                        last_slab = (b, h, qlo, qw) == slabs[-1]
                        if kt in KT_DVE or (last_slab and kt in (11, 15)):
                            nc.vector.tensor_tensor(
                                out=pt[:, kt, 0:qw],
                                in0=e_sb[:, 0:qw],
                                in1=mask_bf(kt) if kt in KT_DVE else mask_u8(kt),
                                op=mybir.AluOpType.mult,
                            )
                        else:
                            nc.gpsimd.tensor_tensor(
                                out=pt[:, kt, 0:qw],
                                in0=e_sb[:, 0:qw],
                                in1=mask_u8(kt),
                                op=mybir.AluOpType.mult,
                            )
                    if kt in qslot:
                        q_iter(prev, qslot[kt])
                prev = (pt, vb, st, b, h, qlo, qw // 128)

            for qc in range(prev[6]):
                q_iter(prev, qc, tail=True)

    nc.compile()
    _cached["nc"] = nc
    return nc


def kernel(queries, keys, values, mask, _trace=False, **kw):
    queries = np.asarray(queries, dtype=np.float32)
    keys = np.asarray(keys, dtype=np.float32)
    values = np.asarray(values, dtype=np.float32)
    mask = np.asarray(mask, dtype=np.float32)
    nc = build()
    in_maps = []
    for c in range(N_CORES):
        sl = slice(c * B, (c + 1) * B)
        # [V | 1]: ones column so P @ [V|1] emits the denominator
        vp = np.ones((B, KT_TILES, 128, D + 1), dtype=np.float32)
        vp[:, :, :, :D] = values[sl].reshape(B, KT_TILES, 128, D)
        # mask^T packed per (b, h): [B, 2, 128, KT, QH]; then split per
        # k-tile into bf16*255 and u8 copies (one common 255 scale — it
        # cancels in the renormalization)
        mt = (
            mask[sl]
            .transpose(0, 2, 1)  # [B, k, q]
            .reshape(B, KT_TILES, 128, 2, QH)
            .transpose(0, 3, 2, 1, 4)  # [B, 2, 128p, KT, QH]
        )
        mb = (mt[:, :, :, KT_DVE, :] * 255.0).astype(ml_dtypes.bfloat16)
        mu = np.rint(mt[:, :, :, KT_U8, :] * 255.0).astype(np.uint8)
        in_maps.append(
            {
                "queriesT": np.ascontiguousarray(
                    queries[sl].transpose(0, 2, 1) * (SCALE / 16.0)
                ).astype(ml_dtypes.bfloat16),
                "keysT": np.ascontiguousarray(
                    keys[sl].transpose(0, 2, 1)
                ).astype(ml_dtypes.bfloat16),
                "valuesP": np.ascontiguousarray(
                    vp.transpose(0, 2, 1, 3)
                ).astype(ml_dtypes.bfloat16),
                "maskB": np.ascontiguousarray(mb),
                "maskU": np.ascontiguousarray(mu),
            }
        )
    res = run_bass_kernel_spmd(
        nc, in_maps, core_ids=list(range(N_CORES)), trace=_trace
    )
    out = np.concatenate(
        [
            res.results[c]["out"]
            .astype(np.float32)
            .transpose(0, 2, 1, 3)
            .reshape(B, NQ, D)
            for c in range(N_CORES)
        ],
        axis=0,
    )
    if _trace:
        return out, res
    return out
